# Initial kernel scaffold
#
"""SAGAN-style self-attention block on 8 Trainium2 NeuronCores.

Reference computation (per batch element b, C=128, H=W=64, N=4096):
    theta = W_theta @ x_b                       [16, 4096]
    phi   = maxpool2x2(W_phi @ x_b)             [16, 1024]
    g     = maxpool2x2(W_g @ x_b)               [64, 1024]
    S     = theta^T phi                         [4096, 1024]
    beta  = softmax(S, axis=-1)
    o     = g @ beta^T                          [64, 4096]
    out   = gamma * (W_o @ o) + x_b             [128, 4096]

Sharding: data-parallel over batch; core b gets batch element b; weights
replicated; no collectives.

Device dataflow (computes S^T = phi^T theta so softmax's reduction axis
lands on the PE contraction axis; row-sums come for free from a ones
column folded into the g^T stationary operand):

  Conv phase (one pass over the 8 x-chunks of 512 spatial positions):
    pp2 [112, 512] = wgp^T @ x_chunk, where wgp packs [W_g | W_phi | 0 |
    W_phi] so the conv output lands g at rows 0..63 and phi pre-placed at
    quadrant rows 64..79 and 96..111.  One maxpool tensor_reduce per
    chunk produces pg_all [112, 128].  phi for quadrants 0/1 is copied
    ([16,128] per chunk) into phi01; g^T enters gTa via PE transposes +
    ScalarE copies (ACT is idle here).  gTa block = [1 | 0*63 | g^T].

  Attention loop over 8 n-chunks of 512:
    theta [112, 512]   conv with a quadrant-replicated weight (wt_rep)
    S^T   [128m, 512n] = phi_q^T theta_q, 4 concurrent K=16 matmuls via
                         tile_position row groups
    E^T = exp(S^T)     ACT, bf16 out (no max subtraction: |S| <= ~12)
    po [128, 512] = sum_m gTa_m^T @ E^T_m -> row 0 = s_n (softmax
                    denominator), rows 64..127 = unnormalized o
    rcp = 1/s on the [1,512] sums row only; broadcast via ones-matmul
    po2 = (gamma*W_o) @ o  (gamma folded into the weight host-side)
    out = po2 * rcp_bcast + x   (DVE stt + GPSIMD add; the residual
          reads the f32r x upload bitcast as f32 -- saves a second
          2MB HBM read, costing ~1e-4 relative error)

Matmul operands use the FP32R format (fp32 with mantissa rounded to 11
bits; full-rate PE streaming vs 1/4-rate fp32). Host inputs are
pre-rounded; on-device producers write float32r APs so the engines round
on the write port (walrus checkMatmultFP32r requires rounded producers).
"""

import os
import numpy as np

MM_MODE = os.environ.get("K_MM_MODE", "f32r")  # f32r | f32
ET_BF16 = os.environ.get("K_ET_BF16", "1") == "1"  # bf16 attention weights
# bench-only bisection variants (numerically wrong except "full"):
#   notail — skip rcp/broadcast/stt, t1 = copy(po2)
VARIANT = os.environ.get("K_VARIANT", "full")
N_CORES = 8
C = 128
N = 4096       # H*W
M = 1024       # N/4
NCH = 8        # n-chunks
CHUNK = 512


def _round_fp32r(a: np.ndarray) -> np.ndarray:
    """Round fp32 to the FP32R grid (11-bit mantissa, round-half-even)."""
    u = np.ascontiguousarray(a, dtype=np.float32).view(np.uint32)
    lsb = (u >> np.uint32(12)) & np.uint32(1)
    r = (u + np.uint32(0x7FF) + lsb) & np.uint32(0xFFFFF000)
    return r.view(np.float32)


def _build(reps: int = 1):
    from contextlib import nullcontext
    import concourse.bass as bass
    import concourse.tile as tile
    from concourse import bacc, mybir

    f32 = mybir.dt.float32
    fmm = mybir.dt.float32r if MM_MODE == "f32r" else f32
    fet = mybir.dt.bfloat16 if ET_BF16 else fmm
    ts = bass.ts
    ALU = mybir.AluOpType
    ACTF = mybir.ActivationFunctionType

    nc = bacc.Bacc(
        "TRN2", target_bir_lowering=False, debug=False, enable_asserts=False,
        num_devices=N_CORES,
    )
    xr_d = nc.dram_tensor("xr", [C, N], fmm, kind="ExternalInput")
    # all matmul weights packed in one DMA:
    #   cols 0:112   wt_rep  (W_theta^T replicated at quadrant offsets)
    #   cols 112:224 wgp     ([W_g | W_phi | 0 | W_phi] -> conv2 rows:
    #                         g 0:64, phi 64:80, junk 80:96, phi 96:112)
    #   cols 224:352 wo_t    (rows 64:128 = (gamma*W_o)^T)
    wcat_d = nc.dram_tensor("wcat", [128, 352], fmm, kind="ExternalInput")
    id_d = nc.dram_tensor("ident", [64, 64], fmm, kind="ExternalInput")
    out_d = nc.dram_tensor("out", [C, N], f32, kind="ExternalOutput")

    if reps > 1 and reps % 3 == 0:
        UNROLL = 3
    elif reps > 1 and reps % 2 == 0:
        UNROLL = 2
    else:
        UNROLL = 1

    with tile.TileContext(nc) as tc:
        with (
            tc.tile_pool(name="persist", bufs=1) as persist,
            tc.tile_pool(name="dbl", bufs=2) as dbl,
            tc.tile_pool(name="theta", bufs=2) as thpool,
            tc.tile_pool(name="et", bufs=8) as etp,
            tc.tile_pool(name="work", bufs=2) as work,
            tc.tile_pool(name="outp", bufs=3) as outpool,
            tc.tile_pool(name="pspair", bufs=2, space="PSUM") as pspair,
            tc.tile_pool(name="psth", bufs=1, space="PSUM") as psth,
            tc.tile_pool(name="psacc", bufs=1, space="PSUM") as psacc,
            tc.tile_pool(name="pssm", bufs=2, space="PSUM") as pssm,
        ):
          # ---- loop-invariant constants (init once, even when benching
          # with a reps-loop: only gTa cols 64:128 per block are rewritten
          # inside the loop) ---------------------------------------------
          ones_f32 = persist.tile([1, 128], f32, name="ones_f32")
          nc.vector.memset(ones_f32, 1.0)
          ones_sb = persist.tile([1, 128], fmm, name="ones_sb")
          nc.vector.tensor_copy(ones_sb, ones_f32)
          gTa = persist.tile([128, 8 * 128], fet, name="gTa")
          nc.vector.memset(gTa, 0.0)
          nc.vector.memset(
              gTa[:, :].rearrange("p (b c) -> p b c", c=128)[:, :, 0:1], 1.0
          )
          id_sb = persist.tile([64, 64], fmm, name="id_sb")
          nc.sync.dma_start(id_sb, id_d[:, :])

          # ---- double-slot input tiles, loads software-pipelined across
          # the back-edge: body k's inputs are DMA'd during body 1-k's
          # compute (the trailing load drains before the barrier), so no
          # input latency is exposed after the barrier ------------------
          NSLOT = max(UNROLL, 1)
          xr_tiles = [persist.tile([C, N], fmm, name=f"XrS{i}") for i in range(NSLOT)]
          wcat_tiles = [
              persist.tile([128, 352], fmm, name=f"wcatS{i}") for i in range(NSLOT)
          ]

          def load_inputs(slot):
              # weights first: the first conv matmul needs wcat
              nc.sync.dma_start(wcat_tiles[slot], wcat_d[:, :])
              for k in range(NCH):
                  nc.sync.dma_start(
                      xr_tiles[slot][:, ts(k, CHUNK)], xr_d[:, ts(k, CHUNK)]
                  )

          load_inputs(0)

          loop_cm = (
              tc.For_i(
                  0, reps // UNROLL, 1,
                  # SP/Pool bodies are short (<1 IRAM block) — branch
                  # hints there are a net per-edge loss, so hint only the
                  # large-body engines
                  hint_engines=(
                      mybir.EngineType.PE,
                      mybir.EngineType.DVE,
                      mybir.EngineType.Activation,
                  ),
              )
              if reps > 1
              else nullcontext()
          )
          with loop_cm:
           # Two kernel bodies per loop iteration (bufs=2 pools let body
           # k+1's conv prefix overlap body k's attention tail; halves the
           # ~2us barrier cost). Body 1's inputs load during body 0; body
           # 0's NEXT-iteration inputs load during body 1 (trailing).
           for _s in range(1, UNROLL):
               load_inputs(_s)
           for _u in range(UNROLL):
            Xr = xr_tiles[_u]
            wcat = wcat_tiles[_u]
            Xf = Xr.bitcast(f32)
            wt_sb = wcat[:, 0:112]
            wgp_sb = wcat[:, 112:224]
            wo_sb = wcat[:, 224:352]

            # pg_all rows: 0:64 g, 64:80 phi (quadrant 2), 96:112 phi
            # (quadrant 3); phi01 rows 0:16 / 32:48 phi (quadrants 0/1)
            pg_all = dbl.tile([112, M], fmm, name="pg_all", tag="pg_all")
            phi01 = dbl.tile([48, M], fmm, name="phi01", tag="phi01")

            # ---- conv phase: g + phi, pooling, g^T ----------------------
            # chunk PAIRS: 2 conv matmuls into one 2-bank tile, ONE maxpool
            # reduce of FD=1024 — halves the DVE reduce / ACT copy
            # instruction count in the prefix (x is prefetched, so the
            # coarser data dependency costs nothing)
            for p in range(NCH // 2):
                pp2 = pspair.tile([112, 2 * CHUNK], f32, name="pp2", tag="pair")
                for h in range(2):
                    nc.tensor.matmul(
                        pp2[:, h * CHUNK : (h + 1) * CHUNK],
                        wgp_sb,
                        Xr[:, ts(2 * p + h, CHUNK)],
                        start=True,
                        stop=True,
                    )
                nc.vector.tensor_reduce(
                    out=pg_all[:, ts(p, 256)].rearrange("p (i j) -> p i j", i=8, j=32),
                    in_=pp2.rearrange(
                        "p (i di j dj) -> p i j di dj", i=8, di=2, j=32, dj=2
                    ),
                    axis=mybir.AxisListType.XY,
                    op=ALU.max,
                )
                # phi for quadrants 0/1 (on ACT, idle until the exps start)
                nc.scalar.copy(phi01[0:16, ts(p, 256)], pg_all[64:80, ts(p, 256)])
                nc.scalar.copy(phi01[32:48, ts(p, 256)], pg_all[64:80, ts(p, 256)])
                # g^T via PE transpose; ACT is idle here, it does the copy
                for mi in (2 * p, 2 * p + 1):
                    ptr = pssm.tile([128, 64], fmm, name="ptr", tag="small")
                    nc.tensor.transpose(ptr, pg_all[0:64, ts(mi, 128)], id_sb)
                    nc.scalar.copy(gTa[:, mi * 128 + 64 : mi * 128 + 128], ptr)

            def phi_q(j, mi):
                blk = ts(mi, 128)
                if j == 0:
                    return phi01[0:16, blk]
                if j == 1:
                    return phi01[32:48, blk]
                if j == 2:
                    return pg_all[64:80, blk]
                return pg_all[96:112, blk]

            # ---- attention over n-chunks -------------------------------
            # The normalize tail of chunk ci is emitted during ci+1, after
            # the next chunk's theta-conv/S^T/po matmuls: its pbc/po2
            # matmuls wait on DVE results, and the PE queue is strict
            # FIFO — emitting them in-chunk head-of-line blocks the PE
            # (and so the exps that feed the bottleneck ACT engine).
            outp_box = [None]

            def emit_tail(st):
                ci, po, o_sb = st
                rcp_f = work.tile([1, CHUNK], f32, name="rcp_f", tag="rcpf")
                nc.vector.reciprocal_approx_fast(rcp_f, o_sb[0:1, :].bitcast(f32))
                # the f32->fmm cast of 1/s lives here (not in the chunk
                # body): it is only needed by pbc just below, and emitting
                # it earlier would sit ahead of the next chunk's theta
                # copy in the DVE FIFO, lengthening the inter-chunk gate
                rcp1 = work.tile([1, CHUNK], fmm, name="rcp1", tag="rcp")
                nc.vector.tensor_copy(rcp1, rcp_f)
                pbc = pssm.tile([128, CHUNK], f32, name="pbc", tag="small")
                nc.tensor.matmul(pbc, ones_sb, rcp1, start=True, stop=True)
                po2 = pssm.tile([128, CHUNK], f32, name="po2", tag="small")
                nc.tensor.matmul(
                    po2,
                    wo_sb[64:128, :],
                    o_sb[64:128, :],
                    start=True,
                    stop=True,
                    tile_position=(64, 0),
                )
                rbc = work.tile([128, CHUNK], f32, name="rbc", tag="rbc")
                nc.vector.tensor_copy(rbc, pbc)
                t1 = work.tile([128, CHUNK], f32, name="t1", tag="t1")
                nc.vector.scalar_tensor_tensor(
                    t1, in0=po2, scalar=1.0, in1=rbc,
                    op0=ALU.mult, op1=ALU.mult,
                )
                if ci % 2 == 0:
                    outp_box[0] = outpool.tile(
                        [128, 2 * CHUNK], f32, name="outp", tag="out"
                    )
                outp = outp_box[0]
                half = outp[:, (ci % 2) * CHUNK : (ci % 2) * CHUNK + CHUNK]
                nc.gpsimd.tensor_add(half, t1, Xf[:, ts(ci, CHUNK)])
                if ci % 2 == 1:
                    # store via the GPSIMD SWDGE queue: keeps the SP HWDGE
                    # ring free for input loads
                    nc.gpsimd.dma_start(
                        out_d[:, bass.ds((ci - 1) * CHUNK, 2 * CHUNK)], outp
                    )

            pending = None
            for ci in range(NCH):
                thp = psth.tile([112, CHUNK], f32, name="thp", tag="th")
                nc.tensor.matmul(thp, wt_sb, Xr[:, ts(ci, CHUNK)], start=True, stop=True)
                theta = thpool.tile([112, CHUNK], fmm, name="theta", tag="theta")
                nc.vector.tensor_copy(theta, thp)
                # tail of chunk ci-1 emits HERE (top of chunk ci), not at
                # the chunk end: its inputs (rcp1/o_sb of ci-1) are already
                # ready, so it streams without stalls — and the next
                # chunk's theta copy lands ahead of rbc/stt in the DVE
                # FIFO, cutting ~1.3us off the po->next-S^T chain that
                # gates the bottleneck ACT engine each chunk.
                if pending is not None:
                    emit_tail(pending)
                    pending = None

                ets = []
                for q in range(2):
                    pair_a = pspair.tile([128, 1024], f32, name="pair_a", tag="pair")
                    pair_b = pspair.tile([128, 1024], f32, name="pair_b", tag="pair")
                    for j in range(4):
                        mi = 4 * q + j
                        dst = (pair_a if j < 2 else pair_b)[
                            :, (j % 2) * CHUNK : (j % 2) * CHUNK + CHUNK
                        ]
                        nc.tensor.matmul(
                            dst,
                            phi_q(j, mi),
                            theta[32 * j : 32 * j + 16, :],
                            start=True,
                            stop=True,
                            tile_position=(32 * j, 0),
                        )
                    for pair in (pair_a, pair_b):
                        et = etp.tile([128, 1024], fet, name="et", tag="et")
                        nc.scalar.activation(et, pair, ACTF.Exp)
                        ets.append(et)

                po = psacc.tile([128, CHUNK], f32, name="po", tag="acc")
                for mi in range(8):
                    rhs = ets[mi // 2][:, (mi % 2) * CHUNK : (mi % 2) * CHUNK + CHUNK]
                    nc.tensor.matmul(
                        po,
                        gTa[:, mi * 128 : (mi + 1) * 128],
                        rhs,
                        start=(mi == 0),
                        stop=(mi == 7),
                    )

                # full-height po copy (FD-bound, same cost) — brings the s
                # row to SBUF so the reciprocal moves into the tail; the
                # po bank is still released in-chunk
                o_sb = work.tile([128, CHUNK], fmm, name="o_sb", tag="osb")
                nc.vector.tensor_copy(o_sb, po)

                pending = (ci, po, o_sb)
            emit_tail(pending)
           # prefetch next iteration's body-0 inputs; the back-edge drain
           # completes the DMA before the next iteration reads it
           if UNROLL > 1:
               load_inputs(0)

    nc.compile()
    return nc


def _host_prep(x, W_theta, W_phi, W_g, W_o, gamma):
    B = np.asarray(x).shape[0]
    rnd = _round_fp32r if MM_MODE == "f32r" else (lambda a: np.asarray(a, np.float32))
    wcat = np.zeros((128, 352), dtype=np.float32)
    for j in range(4):
        wcat[:, 32 * j : 32 * j + 16] = np.asarray(W_theta, np.float32).T
    wcat[:, 112:176] = np.asarray(W_g, np.float32).T
    wcat[:, 176:192] = np.asarray(W_phi, np.float32).T
    wcat[:, 208:224] = np.asarray(W_phi, np.float32).T
    wcat[64:128, 224:352] = float(gamma) * np.asarray(W_o, np.float32).T
    wcat = rnd(wcat)
    ident = np.eye(64, dtype=np.float32)
    xr = rnd(np.ascontiguousarray(np.asarray(x, dtype=np.float32)))
    in_maps = []
    for b in range(B):
        in_maps.append(
            {
                "xr": np.ascontiguousarray(xr[b].reshape(C, N)),
                "wcat": wcat,
                "ident": ident,
            }
        )
    return in_maps


def run(x, W_theta, W_phi, W_g, W_o, gamma, trace=False, **trace_kwargs):
    from concourse.bass_utils import run_bass_kernel_spmd

    nc = _build()
    in_maps = _host_prep(x, W_theta, W_phi, W_g, W_o, gamma)
    res = run_bass_kernel_spmd(
        nc, in_maps, core_ids=list(range(N_CORES)), trace=trace, **trace_kwargs
    )
    outs = [res.results[b]["out"].reshape(C, 64, 64) for b in range(N_CORES)]
    return np.stack(outs).astype(np.float32), res


def kernel(x, W_theta, W_phi, W_g, W_o, gamma):
    out, _ = run(x, W_theta, W_phi, W_g, W_o, gamma)
    return out



# revision 1
# speedup vs baseline: 2.4856x; 2.4856x over previous
"""SAGAN-style self-attention block on 8 Trainium2 NeuronCores.

Reference computation (per batch element b, C=128, H=W=64, N=4096):
    theta = W_theta @ x_b                       [16, 4096]
    phi   = maxpool2x2(W_phi @ x_b)             [16, 1024]
    g     = maxpool2x2(W_g @ x_b)               [64, 1024]
    S     = theta^T phi                         [4096, 1024]
    beta  = softmax(S, axis=-1)
    o     = g @ beta^T                          [64, 4096]
    out   = gamma * (W_o @ o) + x_b             [128, 4096]

Sharding: data-parallel over batch; core b gets batch element b; weights
replicated; no collectives.

Device dataflow (computes S^T = phi^T theta so softmax's reduction axis
lands on the PE contraction axis; row-sums come for free from a ones
column folded into the g^T stationary operand):

  Conv phase (one pass over the 8 x-chunks of 512 spatial positions):
    pp2 [112, 512] = wgp^T @ x_chunk, where wgp packs [W_g | W_phi | 0 |
    W_phi] so the conv output lands g at rows 0..63 and phi pre-placed at
    quadrant rows 64..79 and 96..111.  One maxpool tensor_reduce per
    chunk produces pg_all [112, 128].  phi for quadrants 0/1 is copied
    ([16,128] per chunk) into phi01; g^T enters gTa via PE transposes +
    ScalarE copies (ACT is idle here).  gTa block = [1 | 0*63 | g^T].

  Attention loop over 8 n-chunks of 512:
    theta [112, 512]   conv with a quadrant-replicated weight (wt_rep)
    S^T   [128m, 512n] = phi_q^T theta_q, 4 concurrent K=16 matmuls via
                         tile_position row groups
    E^T = exp(S^T)     ACT, bf16 out (no max subtraction: |S| <= ~12)
    po [128, 512] = sum_m gTa_m^T @ E^T_m -> row 0 = s_n (softmax
                    denominator), rows 64..127 = unnormalized o
    rcp = 1/s on the [1,512] sums row only; broadcast via ones-matmul
    po2 = (gamma*W_o) @ o  (gamma folded into the weight host-side)
    out = po2 * rcp_bcast + x   (DVE stt + GPSIMD add; the residual
          reads the f32r x upload bitcast as f32 -- saves a second
          2MB HBM read, costing ~1e-4 relative error)

Matmul operands use the FP32R format (fp32 with mantissa rounded to 11
bits; full-rate PE streaming vs 1/4-rate fp32). Host inputs are
pre-rounded; on-device producers write float32r APs so the engines round
on the write port (walrus checkMatmultFP32r requires rounded producers).
"""

import os
import numpy as np

MM_MODE = os.environ.get("K_MM_MODE", "f32r")  # f32r | f32
ET_BF16 = os.environ.get("K_ET_BF16", "1") == "1"  # bf16 attention weights
# bench-only bisection variants (numerically wrong except "full"):
#   notail — skip rcp/broadcast/stt, t1 = copy(po2)
VARIANT = os.environ.get("K_VARIANT", "full")
N_CORES = 8
C = 128
N = 4096       # H*W
M = 1024       # N/4
NCH = 8        # n-chunks
CHUNK = 512


def _round_fp32r(a: np.ndarray) -> np.ndarray:
    """Round fp32 to the FP32R grid (11-bit mantissa, round-half-even)."""
    u = np.ascontiguousarray(a, dtype=np.float32).view(np.uint32)
    lsb = (u >> np.uint32(12)) & np.uint32(1)
    r = (u + np.uint32(0x7FF) + lsb) & np.uint32(0xFFFFF000)
    return r.view(np.float32)


def _build(reps: int = 1):
    from contextlib import nullcontext
    import concourse.bass as bass
    import concourse.tile as tile
    from concourse import bacc, mybir

    f32 = mybir.dt.float32
    fmm = mybir.dt.float32r if MM_MODE == "f32r" else f32
    fet = mybir.dt.bfloat16 if ET_BF16 else fmm
    ts = bass.ts
    ALU = mybir.AluOpType
    ACTF = mybir.ActivationFunctionType

    nc = bacc.Bacc(
        "TRN2", target_bir_lowering=False, debug=False, enable_asserts=False,
        num_devices=N_CORES,
    )
    xr_d = nc.dram_tensor("xr", [C, N], fmm, kind="ExternalInput")
    # all matmul weights packed in one DMA:
    #   cols 0:112   wt_rep  (W_theta^T replicated at quadrant offsets)
    #   cols 112:224 wgp     ([W_g | W_phi | 0 | W_phi] -> conv2 rows:
    #                         g 0:64, phi 64:80, junk 80:96, phi 96:112)
    #   cols 224:352 wo_t    (rows 64:128 = (gamma*W_o)^T)
    wcat_d = nc.dram_tensor("wcat", [128, 352], fmm, kind="ExternalInput")
    id_d = nc.dram_tensor("ident", [64, 64], fmm, kind="ExternalInput")
    out_d = nc.dram_tensor("out", [C, N], f32, kind="ExternalOutput")

    if reps > 1 and reps % 3 == 0:
        UNROLL = 3
    elif reps > 1 and reps % 2 == 0:
        UNROLL = 2
    else:
        UNROLL = 1

    with tile.TileContext(nc) as tc:
        with (
            tc.tile_pool(name="persist", bufs=1) as persist,
            tc.tile_pool(name="dbl", bufs=2) as dbl,
            tc.tile_pool(name="theta", bufs=2) as thpool,
            tc.tile_pool(name="et", bufs=8) as etp,
            tc.tile_pool(name="work", bufs=2) as work,
            tc.tile_pool(name="outp", bufs=3) as outpool,
            tc.tile_pool(name="pspair", bufs=2, space="PSUM") as pspair,
            tc.tile_pool(name="psth", bufs=1, space="PSUM") as psth,
            tc.tile_pool(name="psacc", bufs=1, space="PSUM") as psacc,
            tc.tile_pool(name="pssm", bufs=2, space="PSUM") as pssm,
        ):
          # ---- loop-invariant constants (init once, even when benching
          # with a reps-loop: only gTa cols 64:128 per block are rewritten
          # inside the loop) ---------------------------------------------
          ones_f32 = persist.tile([1, 128], f32, name="ones_f32")
          nc.vector.memset(ones_f32, 1.0)
          ones_sb = persist.tile([1, 128], fmm, name="ones_sb")
          nc.vector.tensor_copy(ones_sb, ones_f32)
          gTa = persist.tile([128, 8 * 128], fet, name="gTa")
          nc.vector.memset(gTa, 0.0)
          nc.vector.memset(
              gTa[:, :].rearrange("p (b c) -> p b c", c=128)[:, :, 0:1], 1.0
          )
          id_sb = persist.tile([64, 64], fmm, name="id_sb")
          nc.sync.dma_start(id_sb, id_d[:, :])

          # ---- double-slot input tiles, loads software-pipelined across
          # the back-edge: body k's inputs are DMA'd during body 1-k's
          # compute (the trailing load drains before the barrier), so no
          # input latency is exposed after the barrier ------------------
          NSLOT = max(UNROLL, 1)
          xr_tiles = [persist.tile([C, N], fmm, name=f"XrS{i}") for i in range(NSLOT)]
          wcat_tiles = [
              persist.tile([128, 352], fmm, name=f"wcatS{i}") for i in range(NSLOT)
          ]

          def load_inputs(slot):
              # weights first: the first conv matmul needs wcat
              nc.sync.dma_start(wcat_tiles[slot], wcat_d[:, :])
              for k in range(NCH):
                  nc.sync.dma_start(
                      xr_tiles[slot][:, ts(k, CHUNK)], xr_d[:, ts(k, CHUNK)]
                  )

          load_inputs(0)

          loop_cm = (
              tc.For_i(
                  0, reps // UNROLL, 1,
                  # SP/Pool bodies are short (<1 IRAM block) — branch
                  # hints there are a net per-edge loss, so hint only the
                  # large-body engines
                  hint_engines=(
                      mybir.EngineType.PE,
                      mybir.EngineType.DVE,
                      mybir.EngineType.Activation,
                  ),
              )
              if reps > 1
              else nullcontext()
          )
          with loop_cm:
           # Two kernel bodies per loop iteration (bufs=2 pools let body
           # k+1's conv prefix overlap body k's attention tail; halves the
           # ~2us barrier cost). Body 1's inputs load during body 0; body
           # 0's NEXT-iteration inputs load during body 1 (trailing).
           for _s in range(1, UNROLL):
               load_inputs(_s)
           for _u in range(UNROLL):
            Xr = xr_tiles[_u]
            wcat = wcat_tiles[_u]
            Xf = Xr.bitcast(f32)
            wt_sb = wcat[:, 0:112]
            wgp_sb = wcat[:, 112:224]
            wo_sb = wcat[:, 224:352]

            # pg_all rows: 0:64 g, 64:80 phi (quadrant 2), 96:112 phi
            # (quadrant 3); phi01 rows 0:16 / 32:48 phi (quadrants 0/1)
            pg_all = dbl.tile([112, M], fmm, name="pg_all", tag="pg_all")
            phi01 = dbl.tile([48, M], fmm, name="phi01", tag="phi01")

            # ---- conv phase: g + phi, pooling, g^T ----------------------
            # chunk PAIRS: 2 conv matmuls into one 2-bank tile, ONE maxpool
            # reduce of FD=1024 — halves the DVE reduce / ACT copy
            # instruction count in the prefix (x is prefetched, so the
            # coarser data dependency costs nothing)
            for p in range(NCH // 2):
                pp2 = pspair.tile([112, 2 * CHUNK], f32, name="pp2", tag="pair")
                for h in range(2):
                    nc.tensor.matmul(
                        pp2[:, h * CHUNK : (h + 1) * CHUNK],
                        wgp_sb,
                        Xr[:, ts(2 * p + h, CHUNK)],
                        start=True,
                        stop=True,
                    )
                nc.vector.tensor_reduce(
                    out=pg_all[:, ts(p, 256)].rearrange("p (i j) -> p i j", i=8, j=32),
                    in_=pp2.rearrange(
                        "p (i di j dj) -> p i j di dj", i=8, di=2, j=32, dj=2
                    ),
                    axis=mybir.AxisListType.XY,
                    op=ALU.max,
                )
                # phi for quadrants 0/1 (on ACT, idle until the exps start)
                nc.scalar.copy(phi01[0:16, ts(p, 256)], pg_all[64:80, ts(p, 256)])
                nc.scalar.copy(phi01[32:48, ts(p, 256)], pg_all[64:80, ts(p, 256)])
                # g^T via PE transpose; ACT is idle here, it does the copy
                for mi in (2 * p, 2 * p + 1):
                    ptr = pssm.tile([128, 64], fmm, name="ptr", tag="small")
                    nc.tensor.transpose(ptr, pg_all[0:64, ts(mi, 128)], id_sb)
                    nc.scalar.copy(gTa[:, mi * 128 + 64 : mi * 128 + 128], ptr)

            def phi_q(j, mi):
                blk = ts(mi, 128)
                if j == 0:
                    return phi01[0:16, blk]
                if j == 1:
                    return phi01[32:48, blk]
                if j == 2:
                    return pg_all[64:80, blk]
                return pg_all[96:112, blk]

            # ---- attention over n-chunks -------------------------------
            # The normalize tail of chunk ci is emitted during ci+1, after
            # the next chunk's theta-conv/S^T/po matmuls: its pbc/po2
            # matmuls wait on DVE results, and the PE queue is strict
            # FIFO — emitting them in-chunk head-of-line blocks the PE
            # (and so the exps that feed the bottleneck ACT engine).
            outp_box = [None]

            def emit_tail(st):
                ci, po, o_sb = st
                rcp_f = work.tile([1, CHUNK], f32, name="rcp_f", tag="rcpf")
                nc.vector.reciprocal_approx_fast(rcp_f, o_sb[0:1, :].bitcast(f32))
                # the f32->fmm cast of 1/s lives here (not in the chunk
                # body): it is only needed by pbc just below, and emitting
                # it earlier would sit ahead of the next chunk's theta
                # copy in the DVE FIFO, lengthening the inter-chunk gate
                rcp1 = work.tile([1, CHUNK], fmm, name="rcp1", tag="rcp")
                nc.vector.tensor_copy(rcp1, rcp_f)
                pbc = pssm.tile([128, CHUNK], f32, name="pbc", tag="small")
                nc.tensor.matmul(pbc, ones_sb, rcp1, start=True, stop=True)
                po2 = pssm.tile([128, CHUNK], f32, name="po2", tag="small")
                nc.tensor.matmul(
                    po2,
                    wo_sb[64:128, :],
                    o_sb[64:128, :],
                    start=True,
                    stop=True,
                    tile_position=(64, 0),
                )
                rbc = work.tile([128, CHUNK], f32, name="rbc", tag="rbc")
                nc.vector.tensor_copy(rbc, pbc)
                t1 = work.tile([128, CHUNK], f32, name="t1", tag="t1")
                nc.vector.scalar_tensor_tensor(
                    t1, in0=po2, scalar=1.0, in1=rbc,
                    op0=ALU.mult, op1=ALU.mult,
                )
                if ci % 2 == 0:
                    outp_box[0] = outpool.tile(
                        [128, 2 * CHUNK], f32, name="outp", tag="out"
                    )
                outp = outp_box[0]
                half = outp[:, (ci % 2) * CHUNK : (ci % 2) * CHUNK + CHUNK]
                nc.gpsimd.tensor_add(half, t1, Xf[:, ts(ci, CHUNK)])
                if ci % 2 == 1:
                    # store via the GPSIMD SWDGE queue: keeps the SP HWDGE
                    # ring free for input loads
                    nc.gpsimd.dma_start(
                        out_d[:, bass.ds((ci - 1) * CHUNK, 2 * CHUNK)], outp
                    )

            pending = None
            for ci in range(NCH):
                thp = psth.tile([112, CHUNK], f32, name="thp", tag="th")
                nc.tensor.matmul(thp, wt_sb, Xr[:, ts(ci, CHUNK)], start=True, stop=True)
                theta = thpool.tile([112, CHUNK], fmm, name="theta", tag="theta")
                nc.vector.tensor_copy(theta, thp)
                # tail of chunk ci-1 emits HERE (top of chunk ci), not at
                # the chunk end: its inputs (rcp1/o_sb of ci-1) are already
                # ready, so it streams without stalls — and the next
                # chunk's theta copy lands ahead of rbc/stt in the DVE
                # FIFO, cutting ~1.3us off the po->next-S^T chain that
                # gates the bottleneck ACT engine each chunk.
                if pending is not None:
                    emit_tail(pending)
                    pending = None

                ets = []
                for q in range(2):
                    pair_a = pspair.tile([128, 1024], f32, name="pair_a", tag="pair")
                    pair_b = pspair.tile([128, 1024], f32, name="pair_b", tag="pair")
                    for j in range(4):
                        mi = 4 * q + j
                        dst = (pair_a if j < 2 else pair_b)[
                            :, (j % 2) * CHUNK : (j % 2) * CHUNK + CHUNK
                        ]
                        nc.tensor.matmul(
                            dst,
                            phi_q(j, mi),
                            theta[32 * j : 32 * j + 16, :],
                            start=True,
                            stop=True,
                            tile_position=(32 * j, 0),
                        )
                    for pair in (pair_a, pair_b):
                        et = etp.tile([128, 1024], fet, name="et", tag="et")
                        nc.scalar.activation(et, pair, ACTF.Exp)
                        ets.append(et)

                po = psacc.tile([128, CHUNK], f32, name="po", tag="acc")
                for mi in range(8):
                    rhs = ets[mi // 2][:, (mi % 2) * CHUNK : (mi % 2) * CHUNK + CHUNK]
                    nc.tensor.matmul(
                        po,
                        gTa[:, mi * 128 : (mi + 1) * 128],
                        rhs,
                        start=(mi == 0),
                        stop=(mi == 7),
                    )

                # full-height po copy (FD-bound, same cost) — brings the s
                # row to SBUF so the reciprocal moves into the tail; the
                # po bank is still released in-chunk
                o_sb = work.tile([128, CHUNK], fmm, name="o_sb", tag="osb")
                nc.vector.tensor_copy(o_sb, po)

                pending = (ci, po, o_sb)
            emit_tail(pending)
           # prefetch next iteration's body-0 inputs; the back-edge drain
           # completes the DMA before the next iteration reads it
           if UNROLL > 1:
               load_inputs(0)

    nc.compile()
    return nc


def _host_prep(x, W_theta, W_phi, W_g, W_o, gamma):
    B = np.asarray(x).shape[0]
    rnd = _round_fp32r if MM_MODE == "f32r" else (lambda a: np.asarray(a, np.float32))
    wcat = np.zeros((128, 352), dtype=np.float32)
    for j in range(4):
        wcat[:, 32 * j : 32 * j + 16] = np.asarray(W_theta, np.float32).T
    wcat[:, 112:176] = np.asarray(W_g, np.float32).T
    wcat[:, 176:192] = np.asarray(W_phi, np.float32).T
    wcat[:, 208:224] = np.asarray(W_phi, np.float32).T
    wcat[64:128, 224:352] = float(gamma) * np.asarray(W_o, np.float32).T
    wcat = rnd(wcat)
    ident = np.eye(64, dtype=np.float32)
    xr = rnd(np.ascontiguousarray(np.asarray(x, dtype=np.float32)))
    in_maps = []
    for b in range(B):
        in_maps.append(
            {
                "xr": np.ascontiguousarray(xr[b].reshape(C, N)),
                "wcat": wcat,
                "ident": ident,
            }
        )
    return in_maps


def run(x, W_theta, W_phi, W_g, W_o, gamma, trace=False, **trace_kwargs):
    from concourse.bass_utils import run_bass_kernel_spmd

    nc = _build()
    in_maps = _host_prep(x, W_theta, W_phi, W_g, W_o, gamma)
    res = run_bass_kernel_spmd(
        nc, in_maps, core_ids=list(range(N_CORES)), trace=trace, **trace_kwargs
    )
    outs = [res.results[b]["out"].reshape(C, 64, 64) for b in range(N_CORES)]
    return np.stack(outs).astype(np.float32), res


def kernel(x, W_theta, W_phi, W_g, W_o, gamma):
    out, _ = run(x, W_theta, W_phi, W_g, W_o, gamma)
    return out

